# revision 1
# baseline (speedup 1.0000x reference)
"""Batched Bjorck orthogonalization (512 x 256 x 256, 7 iters) on 8 TRN2 cores.

Per-matrix recurrence (beta=0.5):
    A = W^T W
    W <- W @ (1.5 I - 0.5 A)

Implementation notes:
  - Batch dim (512) sharded across 8 cores -> 64 matrices/core, no comms.
  - Dual state (W, V=W^T) avoids per-iteration transposes: with
    M = 1.5I - 0.5A,
        A  = W^T W      (lhsT=W chunk,  rhs=W)
        W' = V^T M      (lhsT=V chunk,  rhs=M)   [= W M]
        V' = M^T V      (lhsT=M chunk,  rhs=V)   [= M V = W'^T, M symmetric]
    V0 arrives by DMA from a second host-prepared input "wt" (numpy
    transpose of w, exact) — no transpose matmuls at all.  12 matmuls/iter
    (8 on the last), all [128x128] @ [128x256].
  - Operand dtype float16: same PE rate as bf16 but 10-bit mantissa
    (masked rel err vs fp32 reference ~1.3e-3 vs 8.6e-3 bf16 / 7.2e-4
    fp32r), and unlike fp32r it gets fast-weight-load, so the LDWEIGHTS
    stream (~97 ns) hides under the matmul stream (~112 ns/mm, the PE
    fill-rate limit).  Inputs are rounded to fp16 on the host (halves
    input DMA); PSUM accumulation stays fp32 and the last iteration
    writes an fp32 tile for the output DMA.
  - M built in one fused DVE op per chunk:
    scalar_tensor_tensor(out, A_psum, -0.5, 1.5I); matmul groups k-outer.
  - Each operand is [128, 2, 256] (row chunks); each product accumulates
    in one PSUM bank, one accumulation group per bank.
  - Pipelining: 4 matrices interleaved per group, next group's loads
    prefetched a full group early; input DMA on the sync queue, output on
    the gpsimd queue; state pools 12-deep; PSUM pA=3/pW=3/pV'=2 banks;
    copybacks split between DVE and ACT.
  - Measured on trn2 (8 cores): exec ~615 us, TensorMatrix ~96% busy at
    the N=256 fill rate, PE idle ~6 us.  Matrix 85 of the batch has
    sigma_max > sqrt(5) and diverges to inf/NaN in the fp32 reference
    itself; the kernel reproduces the same non-finite pattern.
"""

import numpy as np

N_CORES = 8
B_FULL = 512
N = 256
NITERS = 7
BETA = 0.5

_CACHE = {}


def _build_nc(n_mats, n_iters=NITERS):
    import concourse.bass as bass  # noqa: F401
    import concourse.mybir as mybir
    from concourse import bacc
    from concourse.tile import TileContext
    from concourse.masks import make_identity
    from concourse.bass import ds

    F32 = mybir.dt.float32
    F32R = mybir.dt.float16  # fp16: same PE rate + FWL, ~1e-3-class precision
    ADD = mybir.AluOpType.add
    MULT = mybir.AluOpType.mult

    nc = bacc.Bacc(None, target_bir_lowering=False)
    w_in = nc.declare_dram_parameter("w", [n_mats, N, N], F32R, isOutput=False)
    wt_in = nc.declare_dram_parameter("wt", [n_mats, N, N], F32R, isOutput=False)
    w_out = nc.declare_dram_parameter("out", [n_mats, N, N], F32, isOutput=True)

    def mm_group(psum, lhs_tile, rhs_tile):
        # psum[:, m, :] = sum_k lhs_tile[:, k, 128m:128m+128]^T @ rhs_tile[:, k, :]
        # k-outer so both m-chunks consume rhs chunk k=0 before k=1 is needed
        n_mm = 0
        for k in range(2):
            for m in range(2):
                nc.tensor.matmul(
                    psum[:, m, :],
                    lhsT=lhs_tile[:, k, ds(128 * m, 128)],
                    rhs=rhs_tile[:, k, :],
                    start=(n_mm == 0),
                    stop=(n_mm == 3),
                )
                n_mm += 1

    with TileContext(nc) as tc:
        with (
            tc.tile_pool(name="const", bufs=1) as cpool,
            tc.tile_pool(name="state", bufs=3) as spool,
            tc.tile_pool(name="psum", bufs=2, space="PSUM") as ppool,
        ):
            id128 = cpool.tile([128, 128], F32, name="id128")
            make_identity(nc, id128)
            idstage = cpool.tile([128, 2, N], F32, name="idstage")
            nc.vector.memset(idstage[:], 0.0)
            nc.vector.tensor_copy(idstage[:, 0, 0:128], id128[:])
            nc.vector.tensor_copy(idstage[:, 1, 128:256], id128[:])
            id15 = cpool.tile([128, 2, N], F32R, name="id15")
            nc.vector.tensor_scalar_mul(id15[:], idstage[:], 1.0 + BETA)

            GROUP = 4  # matrices emitted interleaved, for cross-matrix overlap
            groups = [
                range(g0, min(g0 + GROUP, n_mats))
                for g0 in range(0, n_mats, GROUP)
            ]

            def load(mat):
                Wsb = spool.tile(
                    [128, 2, N], F32R, name=f"W_{mat}", tag="W", bufs=12
                )
                nc.sync.dma_start(
                    Wsb[:], w_in[mat].rearrange("(c p) n -> p c n", p=128)
                )
                Vsb = spool.tile(
                    [128, 2, N], F32R, name=f"V0_{mat}", tag="V", bufs=12
                )
                nc.sync.dma_start(
                    Vsb[:], wt_in[mat].rearrange("(c p) n -> p c n", p=128)
                )
                return Wsb, Vsb

            pending = {mat: load(mat) for mat in groups[0]}
            for gi, mats in enumerate(groups):
                W, V = {}, {}
                for mat in mats:
                    W[mat], V[mat] = pending.pop(mat)
                # prefetch next group's loads while this group computes
                if gi + 1 < len(groups):
                    for mat in groups[gi + 1]:
                        pending[mat] = load(mat)

                for t in range(n_iters):
                    last = t == n_iters - 1
                    for mat in mats:
                        psumA = ppool.tile(
                            [128, 2, N], F32, name=f"pA_{mat}_{t}", tag="pA", bufs=3
                        )
                        mm_group(psumA, W[mat], W[mat])
                        Msb = spool.tile(
                            [128, 2, N], F32R, name=f"M_{mat}_{t}", tag="M", bufs=12
                        )
                        for c in range(2):
                            nc.vector.scalar_tensor_tensor(
                                out=Msb[:, c, :],
                                in0=psumA[:, c, :],
                                scalar=-BETA,
                                in1=id15[:, c, :],
                                op0=MULT,
                                op1=ADD,
                            )
                        psumW = ppool.tile(
                            [128, 2, N], F32, name=f"pW_{mat}_{t}", tag="pW", bufs=3
                        )
                        mm_group(psumW, V[mat], Msb)
                        if last:
                            newW = spool.tile(
                                [128, 2, N],
                                F32,
                                name=f"Wo_{mat}_{t}",
                                tag="Wout",
                                bufs=4,
                            )
                        else:
                            newW = spool.tile(
                                [128, 2, N],
                                F32R,
                                name=f"Wn_{mat}_{t}",
                                tag="W",
                                bufs=12,
                            )
                        nc.scalar.copy(newW[:], psumW[:])
                        if not last:
                            psumV2 = ppool.tile(
                                [128, 2, N],
                                F32,
                                name=f"pV2_{mat}_{t}",
                                tag="pV",
                                bufs=2,
                            )
                            mm_group(psumV2, Msb, V[mat])
                            newV = spool.tile(
                                [128, 2, N],
                                F32R,
                                name=f"Vn_{mat}_{t}",
                                tag="V",
                                bufs=12,
                            )
                            if mat % 2 == 0:
                                nc.scalar.copy(newV[:], psumV2[:])
                            else:
                                nc.vector.tensor_copy(newV[:], psumV2[:])
                            V[mat] = newV
                        W[mat] = newW

                for mat in mats:
                    nc.gpsimd.dma_start(
                        w_out[mat].rearrange("(c p) n -> p c n", p=128), W[mat][:]
                    )
    nc.finalize()
    return nc


def _run_spmd(w, trace=False):
    from concourse.bass_utils import run_bass_kernel_spmd

    w = np.ascontiguousarray(w, dtype=np.float32)
    b = w.shape[0]
    n_mats = b // N_CORES
    key = (n_mats,)
    if key not in _CACHE:
        _CACHE[key] = _build_nc(n_mats)
    nc = _CACHE[key]

    shards = w.reshape(N_CORES, n_mats, N, N).astype(np.float16)
    shards_t = np.ascontiguousarray(shards.transpose(0, 1, 3, 2))
    in_maps = [{"w": shards[i], "wt": shards_t[i]} for i in range(N_CORES)]
    res = run_bass_kernel_spmd(
        nc, in_maps, core_ids=list(range(N_CORES)), trace=trace
    )
    out = np.concatenate([res.results[i]["out"] for i in range(N_CORES)], axis=0)
    return out.reshape(b, N, N).astype(np.float32), res


def kernel(w):
    out, _ = _run_spmd(w, trace=False)
    return out

